# revision 4
# baseline (speedup 1.0000x reference)
"""8-core sharded BertGraphSelfAttention for Trainium2.

Data-parallel over batch b (16 -> 2 per core) with a single jitted
shard_map over all 8 NeuronCores: one dispatch, parallel transfers,
bf16 matmuls with fp32 accumulation, fp32 softmax/bias math.
"""

import math

import numpy as np

H = 4
HD = 128
MAXREL = 16

B, M, SEQ, DIM = 16, 36, 128, 512
N_CORES = 8

_CACHE = {}

# Static Shaw relative-position one-hot: [SEQ, SEQ, 2*MAXREL+1]
_r = np.arange(SEQ)
_dist = np.clip(_r[None, :] - _r[:, None], -MAXREL, MAXREL) + MAXREL


def _build():
    import jax
    import jax.numpy as jnp
    from jax.sharding import Mesh, NamedSharding, PartitionSpec as P
    from jax.experimental.shard_map import shard_map

    devs = jax.devices()[:N_CORES]
    mesh = Mesh(np.asarray(devs), ("b",))
    scale = 1.0 / math.sqrt(HD)

    onehot = jnp.asarray(
        (_dist[:, :, None] == np.arange(2 * MAXREL + 1)[None, None, :]).astype(
            np.float32
        ),
        dtype=jnp.bfloat16,
    )

    bf = jnp.bfloat16
    f32 = jnp.float32

    def f(hs, mask, sg5, Wq_s, bq_s, Wk_s, bk_s, Wv_s, bv_s,
          Wq_t, bq_t, Wk_t, bk_t, Wv_t, bv_t, rel_k, rel_v):
        # local shapes: hs [b, M, SEQ, DIM]; mask [b, M, SEQ]; sg5 [b, SEQ, H, M, M]
        # head-split/merge and seq<->m transposes are folded into the einsums
        # via 4d weight views, so the compiler keeps layout inside dot_general.
        hsb = hs.astype(bf)                              # [b, m, s, i]

        def w4(w):
            return w.astype(bf).reshape(DIM, H, HD)

        def b3(bias):
            return bias.reshape(H, 1, HD)

        # ---- branch 1: graph-masked attention over nodes m ----
        # q[b,s,h,m,d] straight from natural-layout hs
        q = (jnp.einsum("bmsi,ihd->bshmd", hsb, w4(Wq_s),
                        preferred_element_type=f32) + b3(bq_s)).astype(bf)
        k = (jnp.einsum("bmsi,ihd->bshmd", hsb, w4(Wk_s),
                        preferred_element_type=f32) + b3(bk_s)).astype(bf)
        v = (jnp.einsum("bmsi,ihd->bshmd", hsb, w4(Wv_s),
                        preferred_element_type=f32) + b3(bv_s)).astype(bf)
        scores = jnp.einsum("bshqd,bshkd->bshqk", q, k,
                            preferred_element_type=f32) * scale
        mask_t = jnp.swapaxes(mask, 1, 2)                # [b, SEQ, M]
        sg = jnp.where(mask_t[:, :, None, None, :] == 0, 0.0, sg5)
        bias1 = (1.0 - sg) * -10000.0
        probs = jax.nn.softmax(scores + bias1, axis=-1).astype(bf)
        ctx = jnp.einsum("bshqk,bshkd->bshqd", probs, v,
                         preferred_element_type=f32).astype(bf)

        # ---- branch 2: temporal attention with Shaw relative positions ----
        # project ctx[b,s,h,m,d] directly to [b,m,j,q(=s),c]
        def w44(w):
            return w.astype(bf).reshape(H, HD, H, HD)

        q2 = (jnp.einsum("bshmd,hdjc->bmjsc", ctx, w44(Wq_t),
                         preferred_element_type=f32) + b3(bq_t)).astype(bf)
        k2 = (jnp.einsum("bshmd,hdjc->bmjsc", ctx, w44(Wk_t),
                         preferred_element_type=f32) + b3(bk_t)).astype(bf)
        v2 = (jnp.einsum("bshmd,hdjc->bmjsc", ctx, w44(Wv_t),
                         preferred_element_type=f32) + b3(bv_t)).astype(bf)
        rk = jnp.einsum("qkr,rd->qkd", onehot, rel_k.astype(bf),
                        preferred_element_type=f32).astype(bf)
        rv = jnp.einsum("qkr,rd->qkd", onehot, rel_v.astype(bf),
                        preferred_element_type=f32).astype(bf)
        scores2 = jnp.einsum("bmjqc,bmjkc->bmjqk", q2, k2,
                             preferred_element_type=f32)
        scores2 = scores2 + jnp.einsum("bmjqc,qkc->bmjqk", q2, rk,
                                       preferred_element_type=f32)
        scores2 = scores2 * scale
        scores2 = scores2 + (1.0 - mask)[:, :, None, None, :] * -10000.0
        probs2 = jax.nn.softmax(scores2, axis=-1).astype(bf)
        ctx2 = jnp.einsum("bmjqk,bmjkc->bmjqc", probs2, v2,
                          preferred_element_type=f32)
        ctx2 = ctx2 + jnp.einsum("bmjqk,qkc->bmjqc", probs2, rv,
                                 preferred_element_type=f32)
        # [b, M, H, SEQ, HD] -> [b, M, SEQ, DIM]
        out = jnp.moveaxis(ctx2, 2, 3).reshape(ctx2.shape[0], M, SEQ, DIM)
        return out.astype(f32)

    shd = NamedSharding(mesh, P("b"))
    rep = NamedSharding(mesh, P())
    in_specs = (P("b"), P("b"), P("b")) + (P(),) * 14
    fn = jax.jit(
        shard_map(f, mesh=mesh, in_specs=in_specs, out_specs=P("b"),
                  check_rep=False),
        in_shardings=(shd, shd, shd) + (rep,) * 14,
        out_shardings=shd,
    )
    return fn, shd, rep


def kernel(hidden_states, attention_mask, sim_graph,
           Wq_sim, bq_sim, Wk_sim, bk_sim, Wv_sim, bv_sim,
           Wq_seq, bq_seq, Wk_seq, bk_seq, Wv_seq, bv_seq,
           rel_k, rel_v, b=None, m=None, seq=None, dim=None, **_):
    if "fn" not in _CACHE:
        _CACHE["fn"], _CACHE["shd"], _CACHE["rep"] = _build()
    fn = _CACHE["fn"]

    hs = np.ascontiguousarray(np.asarray(hidden_states, np.float32))
    mask = np.ascontiguousarray(np.asarray(attention_mask, np.float32))
    sg5 = np.ascontiguousarray(
        np.asarray(sim_graph, np.float32).reshape(B, SEQ, H, M, M))
    ws = [np.asarray(w, np.float32) for w in
          (Wq_sim, bq_sim, Wk_sim, bk_sim, Wv_sim, bv_sim,
           Wq_seq, bq_seq, Wk_seq, bk_seq, Wv_seq, bv_seq)]
    out = fn(hs, mask, sg5, *ws,
             np.asarray(rel_k, np.float32), np.asarray(rel_v, np.float32))
    return np.asarray(out)


# revision 5
# speedup vs baseline: 1.7824x; 1.7824x over previous
"""8-core sharded BertGraphSelfAttention for Trainium2.

Data-parallel over batch b (16 -> 2 per core) with a single jitted
shard_map over all 8 NeuronCores: one dispatch, parallel transfers,
bf16 matmuls with fp32 accumulation, fp32 softmax/bias math.
"""

import math

import numpy as np

H = 4
HD = 128
MAXREL = 16

B, M, SEQ, DIM = 16, 36, 128, 512
N_CORES = 8

_CACHE = {}

# Static Shaw relative-position one-hot: [SEQ, SEQ, 2*MAXREL+1]
_r = np.arange(SEQ)
_dist = np.clip(_r[None, :] - _r[:, None], -MAXREL, MAXREL) + MAXREL


def _build():
    import jax
    import jax.numpy as jnp
    from jax.sharding import Mesh, NamedSharding, PartitionSpec as P
    from jax.experimental.shard_map import shard_map

    devs = jax.devices()[:N_CORES]
    mesh = Mesh(np.asarray(devs), ("b",))
    scale = 1.0 / math.sqrt(HD)

    onehot = jnp.asarray(
        (_dist[:, :, None] == np.arange(2 * MAXREL + 1)[None, None, :]).astype(
            np.float32
        ),
        dtype=jnp.bfloat16,
    )

    bf = jnp.bfloat16
    f32 = jnp.float32

    def f(hs, mask, sg5, Wq_s, bq_s, Wk_s, bk_s, Wv_s, bv_s,
          Wq_t, bq_t, Wk_t, bk_t, Wv_t, bv_t, rel_k, rel_v):
        # local shapes: hs [b, M, SEQ, DIM]; mask [b, M, SEQ]; sg5 [b, SEQ, H, M, M]
        def proj(x, w, bias):
            # x [..., DIM] bf16 -> [..., DIM] f32 accum -> bf16
            y = jnp.einsum("...i,io->...o", x, w.astype(bf),
                           preferred_element_type=f32)
            return (y + bias).astype(bf)

        def heads(x, L):
            # [..., L, DIM] -> [..., H, L, HD]
            return jnp.moveaxis(
                x.reshape(x.shape[:-2] + (L, H, HD)), -2, -3)

        # ---- branch 1: graph-masked attention over nodes m ----
        hs1 = jnp.swapaxes(hs, 1, 2).astype(bf)          # [b, SEQ, M, DIM]
        q = heads(proj(hs1, Wq_s, bq_s), M)              # [b, SEQ, H, M, HD]
        k = heads(proj(hs1, Wk_s, bk_s), M)
        v = heads(proj(hs1, Wv_s, bv_s), M)
        scores = jnp.einsum("bshqd,bshkd->bshqk", q, k,
                            preferred_element_type=f32) * scale
        mask_t = jnp.swapaxes(mask, 1, 2)                # [b, SEQ, M]
        sg = jnp.where(mask_t[:, :, None, None, :] == 0, 0.0, sg5)
        bias1 = (1.0 - sg) * -10000.0
        probs = jax.nn.softmax(scores + bias1, axis=-1).astype(bf)
        ctx = jnp.einsum("bshqk,bshkd->bshqd", probs, v,
                         preferred_element_type=f32)     # [b, SEQ, H, M, HD]
        # recombine heads -> [b, SEQ, M, DIM]
        ctx = jnp.moveaxis(ctx, 2, 3).reshape(ctx.shape[0], SEQ, M, DIM)

        # ---- branch 2: temporal attention with Shaw relative positions ----
        hs2 = jnp.swapaxes(ctx, 1, 2).astype(bf)         # [b, M, SEQ, DIM]
        q2 = heads(proj(hs2, Wq_t, bq_t), SEQ)           # [b, M, H, SEQ, HD]
        k2 = heads(proj(hs2, Wk_t, bk_t), SEQ)
        v2 = heads(proj(hs2, Wv_t, bv_t), SEQ)
        rk = jnp.einsum("qkr,rd->qkd", onehot, rel_k.astype(bf),
                        preferred_element_type=f32).astype(bf)
        rv = jnp.einsum("qkr,rd->qkd", onehot, rel_v.astype(bf),
                        preferred_element_type=f32).astype(bf)
        scores2 = jnp.einsum("bmhqd,bmhkd->bmhqk", q2, k2,
                             preferred_element_type=f32)
        scores2 = scores2 + jnp.einsum("bmhqd,qkd->bmhqk", q2, rk,
                                       preferred_element_type=f32)
        scores2 = scores2 * scale
        scores2 = scores2 + (1.0 - mask)[:, :, None, None, :] * -10000.0
        probs2 = jax.nn.softmax(scores2, axis=-1).astype(bf)
        ctx2 = jnp.einsum("bmhqk,bmhkd->bmhqd", probs2, v2,
                          preferred_element_type=f32)
        ctx2 = ctx2 + jnp.einsum("bmhqk,qkd->bmhqd", probs2, rv,
                                 preferred_element_type=f32)
        # [b, M, H, SEQ, HD] -> [b, M, SEQ, DIM]
        out = jnp.moveaxis(ctx2, 2, 3).reshape(ctx2.shape[0], M, SEQ, DIM)
        return out.astype(f32)

    shd = NamedSharding(mesh, P("b"))
    rep = NamedSharding(mesh, P())
    in_specs = (P("b"), P("b"), P("b")) + (P(),) * 14
    fn = jax.jit(
        shard_map(f, mesh=mesh, in_specs=in_specs, out_specs=P("b"),
                  check_rep=False),
        in_shardings=(shd, shd, shd) + (rep,) * 14,
        out_shardings=shd,
    )
    return fn, shd, rep


def kernel(hidden_states, attention_mask, sim_graph,
           Wq_sim, bq_sim, Wk_sim, bk_sim, Wv_sim, bv_sim,
           Wq_seq, bq_seq, Wk_seq, bk_seq, Wv_seq, bv_seq,
           rel_k, rel_v, b=None, m=None, seq=None, dim=None, **_):
    if "fn" not in _CACHE:
        _CACHE["fn"], _CACHE["shd"], _CACHE["rep"] = _build()
    fn = _CACHE["fn"]

    hs = np.ascontiguousarray(np.asarray(hidden_states, np.float32))
    mask = np.ascontiguousarray(np.asarray(attention_mask, np.float32))
    sg5 = np.ascontiguousarray(
        np.asarray(sim_graph, np.float32).reshape(B, SEQ, H, M, M))
    ws = [np.asarray(w, np.float32) for w in
          (Wq_sim, bq_sim, Wk_sim, bk_sim, Wv_sim, bv_sim,
           Wq_seq, bq_seq, Wk_seq, bk_seq, Wv_seq, bv_seq)]
    out = fn(hs, mask, sg5, *ws,
             np.asarray(rel_k, np.float32), np.asarray(rel_v, np.float32))
    return np.asarray(out)
